# revision 24
# baseline (speedup 1.0000x reference)
"""Multi-head attention (SuperGlue-style, conv1x1 projections) on 8 Trainium2
NeuronCores.

Sharding: pure data-parallel over batch (B=8 -> 1 batch element per core),
zero collectives. Weights replicated.

Per-core math (one batch element, x* = [D=256, N=2048], H=4 heads, dh=64):
  q = 0.125 * (Wq x + bq)   (score scale folded into q projection)
  k = Wk x + bk
  vT = x^T Wv^T             (v computed transposed: [n, dm] layout)
  per head h:
    S^T[m, n] = k_h[:, m]^T q_h[:, n]        (PE, K=64)
    E = exp(S^T)                              (ScalarE, no max subtraction:
                                               scores ~ N(0,1) so fp32-safe)
    num[d, n]  = sum_m v_aug[m, 65]^T E[m,n]  (PE, K=128; col 64 of v_aug is
                                               ones -> row 64 = softmax denom)
    x_h = num[0:64] * (1/num[64])             (DVE; recip broadcast down
                                               partitions via a K=1 PE
                                               outer-product into PSUM)
  out = Wm' x' + bm_eff     (per-head K=64 accumulation; bv folded into
                             bm_eff = bm + Wm bv since softmax rows sum to 1)

Head channels are interleaved in d_model (dm = i*H + h); all weight
permutations that make heads contiguous are applied on the host for free.

Performance notes (HW-measured on trn2 via axon):
- All matmul operands are float32r (TF32-like): 1 cycle/row on the PE vs 4
  for plain fp32. Rel err vs fp32 reference: 6.2e-4 (fp32 path was 3.7e-6
  but 4x slower on the PE). BIR requires every producer of a f32r-consumed
  tile to write f32r (hence f32r DRAM decls + f32r tile dtypes; memset
  can't write f32r, so the ones column comes from a broadcast copy).
- exp() on ScalarE is the floor: 16.8M elements/core at 1 elem/lane/cycle
  @1.2GHz + 352 cycles/instr overhead -> widest possible activations
  ([128,1024] from 2-bank PSUM tiles) matter.
- Softmax numerator accumulates in PSUM only per 4-m-chunk group, drained
  to an SBUF accumulator on DVE; this frees PSUM for a 3-deep S^T pipeline
  (psS bufs=3 x 2 banks + psN 1 x 2 banks = 8 banks exactly), which was
  worth 2.4x on HW (395us -> 166us steady-state per-core). Replacing the
  reciprocal DRAM-bounce broadcast with a K=1 PE outer product
  (ones[1,64]^T @ recip[1,512] -> PSUM) removed the last DMA latency from
  head boundaries; best quiet-window measurement ~122us/core.
- DMAs are spread across SP/ACT/gpsimd queues; descriptor generation on
  one queue sequencer (~25ns/partition-row) otherwise serializes.
"""

import numpy as np
from contextlib import ExitStack

import concourse.bass as bass
import concourse.tile as tile
from concourse import bacc, mybir
from concourse.bass_utils import run_bass_kernel_spmd

B, D, N, H = 8, 256, 2048, 4
DH = D // H            # 64 per-head channels
PC = 128               # partition chunk
KC = D // PC           # 2 contraction chunks for convs
NT = 512               # free-dim tile (fp32 matmul moving max)
NNT = N // NT          # 4 n-tiles
MC = N // PC           # 16 m-chunks (key/seq chunks on partitions)
VA_W = DH + 1          # 65: per-head v^T columns + ones column
F32 = mybir.dt.float32
F32R = mybir.dt.float32r


def mm(ap):
    """Matmul operands live in float32r tiles (full PE rate; fp32 is 4
    cycles/row). Producers must write f32r-rounded values (BIR rule)."""
    return ap


def emit(ctx: ExitStack, tc: tile.TileContext, io: dict):
    nc = tc.nc
    xq, xk, xv = io["xq"], io["xk"], io["xv"]
    wqT, wkT, wvT, wmT = io["wqT"], io["wkT"], io["wvT"], io["wmT"]
    bq, bk, bm = io["bq"], io["bk"], io["bm"]
    rs = io["rs"]
    out = io["out"]

    consts = ctx.enter_context(tc.tile_pool(name="consts", bufs=1))
    in_pool = ctx.enter_context(tc.tile_pool(name="in_pool", bufs=4))
    qk_pool = ctx.enter_context(tc.tile_pool(name="qk_pool", bufs=2))
    va_pool = ctx.enter_context(tc.tile_pool(name="va_pool", bufs=MC))
    e_pool = ctx.enter_context(tc.tile_pool(name="e_pool", bufs=6))
    x_pool = ctx.enter_context(tc.tile_pool(name="x_pool", bufs=4))
    sm_pool = ctx.enter_context(tc.tile_pool(name="sm_pool", bufs=4))
    bc_pool = ctx.enter_context(tc.tile_pool(name="bc_pool", bufs=4))
    out_pool = ctx.enter_context(tc.tile_pool(name="out_pool", bufs=2))
    psS = ctx.enter_context(tc.tile_pool(name="psS", bufs=3, space="PSUM"))
    psN = ctx.enter_context(tc.tile_pool(name="psN", bufs=1, space="PSUM"))
    acc_pool = ctx.enter_context(tc.tile_pool(name="acc_pool", bufs=4))

    # --- weights / biases to SBUF ---
    w_q = [consts.tile([PC, D], F32R, tag=f"wq{kc}", name="wq") for kc in range(KC)]
    w_k = [consts.tile([PC, D], F32R, tag=f"wk{kc}", name="wk") for kc in range(KC)]
    w_v = [consts.tile([PC, D], F32R, tag=f"wv{kc}", name="wv") for kc in range(KC)]
    for kc in range(KC):
        nc.gpsimd.dma_start(w_q[kc][:], wqT[kc * PC:(kc + 1) * PC, :])
        nc.gpsimd.dma_start(w_k[kc][:], wkT[kc * PC:(kc + 1) * PC, :])
        nc.gpsimd.dma_start(w_v[kc][:], wvT[kc * PC:(kc + 1) * PC, :])
    w_m = [consts.tile([DH, D], F32R, tag=f"wm{h}", name="wm") for h in range(H)]
    for h in range(H):
        nc.gpsimd.dma_start(w_m[h][:], wmT[h * DH:(h + 1) * DH, :])
    ones_sb = consts.tile([PC, 1], F32R, tag="ones", name="ones")
    nc.gpsimd.dma_start(ones_sb[:], io["onec"].partition_broadcast(PC))
    ones_row = consts.tile([1, DH], F32R, tag="onesr", name="onesr")
    nc.vector.tensor_copy(ones_row[:], ones_sb[0:1, 0:1].broadcast_to([1, DH]))
    b_q = [consts.tile([PC, 1], F32, tag=f"bq{oc}", name="bq") for oc in range(KC)]
    b_k = [consts.tile([PC, 1], F32, tag=f"bk{oc}", name="bk") for oc in range(KC)]
    b_m = [consts.tile([PC, 1], F32, tag=f"bm{oc}", name="bm") for oc in range(KC)]
    for oc in range(KC):
        nc.gpsimd.dma_start(b_q[oc][:], bq[oc * PC:(oc + 1) * PC, :])
        nc.gpsimd.dma_start(b_k[oc][:], bk[oc * PC:(oc + 1) * PC, :])
        nc.gpsimd.dma_start(b_m[oc][:], bm[oc * PC:(oc + 1) * PC, :])

    # --- load activations ---
    x_in = {}
    for name, dram, eng in (
        ("xq", xq, nc.sync), ("xk", xk, nc.scalar), ("xv", xv, nc.gpsimd)
    ):
        x_in[name] = [in_pool.tile([PC, N], F32R, tag="xin", name="xin") for _ in range(KC)]
        for kc in range(KC):
            eng.dma_start(x_in[name][kc][:], dram[kc * PC:(kc + 1) * PC, :])

    # --- Q / K projections: out[o', n] = sum_i W^T[i, o'] x[i, n] (+ bias) ---
    q_sb = [qk_pool.tile([PC, N], F32R, tag="qsb", name="qsb") for _ in range(KC)]
    k_sb = [qk_pool.tile([PC, N], F32R, tag="ksb", name="ksb") for _ in range(KC)]
    for w_sb, b_sb, x_sb_in, dst in (
        (w_q, b_q, x_in["xq"], q_sb),
        (w_k, b_k, x_in["xk"], k_sb),
    ):
        for oc in range(KC):
            for nt in range(NNT):
                ps = psS.tile([PC, NT], F32, tag="sps", name="cps")
                for kc in range(KC):
                    nc.tensor.matmul(
                        ps[:],
                        lhsT=mm(w_sb[kc][:, oc * PC:(oc + 1) * PC]),
                        rhs=mm(x_sb_in[kc][:, nt * NT:(nt + 1) * NT]),
                        start=(kc == 0),
                        stop=(kc == KC - 1),
                    )
                nc.vector.tensor_scalar_add(
                    dst[oc][:, nt * NT:(nt + 1) * NT], ps[:], b_sb[oc][:]
                )

    # --- V^T projection + ones column: va[mc] = [128(m), H*65] ---
    va = [va_pool.tile([PC, H * VA_W], F32R, tag="va", name="va") for _ in range(MC)]
    for mc in range(MC):
        ps = psS.tile([PC, D], F32, tag="sps", name="cps")
        for kc in range(KC):
            nc.tensor.matmul(
                ps[:],
                lhsT=mm(x_in["xv"][kc][:, mc * PC:(mc + 1) * PC]),
                rhs=mm(w_v[kc][:]),
                start=(kc == 0),
                stop=(kc == KC - 1),
            )
        ones_cols = va[mc][:].rearrange("p (h w) -> p h w", h=H)[:, :, DH]
        nc.vector.tensor_copy(ones_cols, ones_sb[:].broadcast_to([PC, H]))
        for h in range(H):
            nc.vector.tensor_copy(
                va[mc][:, h * VA_W:h * VA_W + DH],
                ps[:, h * DH:(h + 1) * DH],
            )

    # --- attention per head, n-tiles processed in pairs (1024-wide exp) ---
    x_att = [x_pool.tile([DH, N], F32R, tag="xatt", name="xatt") for _ in range(H)]
    for h in range(H):
        tix = h // 2          # which q/k tile holds this head
        hb = (h % 2) * DH     # partition base of this head inside the tile
        for half in range(2):
            nts = (2 * half, 2 * half + 1)
            GRP = 4
            acc = [acc_pool.tile([VA_W, NT], F32, tag="acc", name="acc")
                   for _ in nts]
            for g in range(MC // GRP):
                nps = psN.tile([VA_W, 2 * NT], F32, tag="nps", name="nps")
                for mc in range(g * GRP, (g + 1) * GRP):
                    sps = psS.tile([PC, 2 * NT], F32, tag="sps", name="sps")
                    for j, nt in enumerate(nts):
                        nc.tensor.matmul(
                            sps[:, j * NT:(j + 1) * NT],
                            lhsT=mm(k_sb[tix][hb:hb + DH, mc * PC:(mc + 1) * PC]),
                            rhs=mm(q_sb[tix][hb:hb + DH, nt * NT:(nt + 1) * NT]),
                            start=True,
                            stop=True,
                        )
                    e_t = e_pool.tile([PC, 2 * NT], F32R, tag="et", name="et")
                    nc.scalar.activation(e_t[:], sps[:],
                                         mybir.ActivationFunctionType.Exp)
                    for j, nt in enumerate(nts):
                        nc.tensor.matmul(
                            nps[:, j * NT:(j + 1) * NT],
                            lhsT=mm(va[mc][:, h * VA_W:(h + 1) * VA_W]),
                            rhs=mm(e_t[:, j * NT:(j + 1) * NT]),
                            start=(mc % GRP == 0),
                            stop=(mc % GRP == GRP - 1),
                        )
                for j in range(2):
                    seg = nps[:, j * NT:(j + 1) * NT]
                    if g == 0:
                        nc.vector.tensor_copy(acc[j][:], seg)
                    else:
                        nc.vector.tensor_add(acc[j][:], acc[j][:], seg)
            ps_b = psN.tile([VA_W, 2 * NT], F32, tag="nps", name="bps")
            for j, nt in enumerate(nts):
                r = sm_pool.tile([1, NT], F32R, tag="recip", name="recip")
                with nc.allow_low_precision(reason="f32r is fp32-width"):
                    nc.vector.reciprocal(r[:], acc[j][DH:DH + 1, :])
                # broadcast recip down 64 partitions: ones[1,64]^T @ r[1,512]
                nc.tensor.matmul(
                    ps_b[0:DH, j * NT:(j + 1) * NT],
                    lhsT=mm(ones_row[:]),
                    rhs=mm(r[:]),
                    start=True,
                    stop=True,
                )
                nc.vector.tensor_mul(
                    x_att[h][:, nt * NT:(nt + 1) * NT],
                    acc[j][0:DH, :],
                    ps_b[0:DH, j * NT:(j + 1) * NT],
                )

    # --- merge projection: out[o, n] = sum_h Wm'^T[h] x_h (+ bm_eff) ---
    for oc in range(KC):
        o_t = out_pool.tile([PC, N], F32, tag="ot", name="ot")
        for nt in range(NNT):
            ps = psS.tile([PC, NT], F32, tag="sps", name="cps")
            for h in range(H):
                nc.tensor.matmul(
                    ps[:],
                    lhsT=mm(w_m[h][:, oc * PC:(oc + 1) * PC]),
                    rhs=mm(x_att[h][:, nt * NT:(nt + 1) * NT]),
                    start=(h == 0),
                    stop=(h == H - 1),
                )
            nc.vector.tensor_scalar_add(
                o_t[:, nt * NT:(nt + 1) * NT], ps[:], b_m[oc][:]
            )
        nc.sync.dma_start(out[oc * PC:(oc + 1) * PC, :], o_t[:])


def build_nc(reps=1):
    nc = bacc.Bacc("TRN2", target_bir_lowering=False, debug=False, num_devices=B)
    io = {
        "xq": nc.dram_tensor("xq", [D, N], F32R, kind="ExternalInput").ap(),
        "xk": nc.dram_tensor("xk", [D, N], F32R, kind="ExternalInput").ap(),
        "xv": nc.dram_tensor("xv", [D, N], F32R, kind="ExternalInput").ap(),
        "wqT": nc.dram_tensor("wqT", [D, D], F32R, kind="ExternalInput").ap(),
        "wkT": nc.dram_tensor("wkT", [D, D], F32R, kind="ExternalInput").ap(),
        "wvT": nc.dram_tensor("wvT", [D, D], F32R, kind="ExternalInput").ap(),
        "wmT": nc.dram_tensor("wmT", [D, D], F32R, kind="ExternalInput").ap(),
        "bq": nc.dram_tensor("bq", [D, 1], F32, kind="ExternalInput").ap(),
        "bk": nc.dram_tensor("bk", [D, 1], F32, kind="ExternalInput").ap(),
        "bm": nc.dram_tensor("bm", [D, 1], F32, kind="ExternalInput").ap(),
        "rs": nc.dram_tensor("rs", [H * NNT, NT], F32).ap(),
        "onec": nc.dram_tensor("onec", [1, 1], F32R, kind="ExternalInput").ap(),
        "out": nc.dram_tensor("out", [D, N], F32, kind="ExternalOutput").ap(),
    }
    with tile.TileContext(nc) as tc:
        if reps == 1:
            with ExitStack() as ctx:
                emit(ctx, tc, io)
        else:
            with tc.For_i(0, reps, 1):
                with ExitStack() as ctx:
                    emit(ctx, tc, io)
    nc.compile()
    return nc


def host_inputs(query, key, value, Wq, bq, Wk, bk, Wv, bv, Wm, bm):
    """Host-side prep: head-deinterleaving permutation + scale/bias folding.

    Returns (shared weight map, list of per-core input maps)."""
    f = np.float32
    t = np.arange(D)
    perm = (t % DH) * H + t // DH  # row t = head-major channel -> original dm

    Wq = np.asarray(Wq, f); Wk = np.asarray(Wk, f); Wv = np.asarray(Wv, f)
    Wm = np.asarray(Wm, f)
    bq = np.asarray(bq, f); bk = np.asarray(bk, f); bv = np.asarray(bv, f)
    bm = np.asarray(bm, f)

    scale = f(1.0 / np.sqrt(DH))
    shared = {
        "onec": np.ones((1, 1), f),
        "wqT": np.ascontiguousarray(Wq.T[:, perm] * scale),
        "wkT": np.ascontiguousarray(Wk.T[:, perm]),
        "wvT": np.ascontiguousarray(Wv.T[:, perm]),
        "wmT": np.ascontiguousarray(Wm.T[perm, :]),
        "bq": np.ascontiguousarray((bq[perm] * scale).reshape(D, 1)),
        "bk": np.ascontiguousarray(bk[perm].reshape(D, 1)),
        "bm": np.ascontiguousarray((bm + Wm @ bv).reshape(D, 1)),
    }
    query = np.asarray(query, f); key = np.asarray(key, f)
    value = np.asarray(value, f)
    in_maps = []
    for b in range(B):
        m = dict(shared)
        m["xq"] = np.ascontiguousarray(query[b])
        m["xk"] = np.ascontiguousarray(key[b])
        m["xv"] = np.ascontiguousarray(value[b])
        in_maps.append(m)
    return shared, in_maps


_NC = None


def get_nc():
    global _NC
    if _NC is None:
        _NC = build_nc()
    return _NC


def kernel(query, key, value, Wq, bq, Wk, bk, Wv, bv, Wm, bm):
    nc = get_nc()
    _, in_maps = host_inputs(query, key, value, Wq, bq, Wk, bk, Wv, bv, Wm, bm)
    res = run_bass_kernel_spmd(nc, in_maps, core_ids=list(range(B)))
    return np.stack([res.results[b]["out"] for b in range(B)], axis=0)
